# revision 1
# baseline (speedup 1.0000x reference)
"""Fused OOQKV attention-with-generated-transform kernel for Trainium2.

Math (per head h):
  g = gelu(x @ Wg_h + bg_h)            # [T, 64, 64] per-token transform
  q,k,v = x @ W{q,k,v}_h + b           # [T, 64]
  qg[t] = q[t] @ g[t]
  att = softmax(qg @ k^T)              # per batch, no scaling
  out_h = att @ v

Sharding: head-parallel, 1 head per core (8 heads, 8 cores); every core
reads the full (host-pre-transposed) xT.

Per-core schedule:
  phase 1 (per 128-token tile): fused q|v_aug|k projection and the
    32768-wide g projection, grouped so consecutive PE matmuls share the
    stationary xT slice (f32r weight switches cost ~2x); biases are K=1
    bf16 matmuls (bf16 keeps them at stream rate; bias magnitudes are
    ~0.04 so bf16 rounding is ~1e-4 absolute). ACT applies exact gelu,
    writing each 512-chunk transposed to (e-major, d-minor) layout so the
    DVE qg contraction multiplies contiguously against a broadcast q view
    and reduces over a contiguous innermost d. PE transposes build kT and
    qgT for phase 2.
  phase 2 (per batch, per 512 query cols): S^T = kT-slice.T @ qgT on PE,
    exp on ACT (no max subtraction; |scores| < 70 so fp32 exp is exact
    enough), then out^T accumulated over m-tiles with v augmented by a
    ones column so row 64 carries the softmax denominator.
Host divides by the denominator row and transposes during the gather.

Matmuls run in float32r (fp32-reduced: 1 cycle/row streaming, ~1e-4
matmul rel err measured on HW); end-to-end rel err vs the fp32 reference
is ~1e-3.
"""

import sys

sys.path.insert(0, "/opt/trn_rl_repo")

import numpy as np

B, N, E, H, D = 4, 1024, 512, 8, 64
T = B * N                 # 4096 flattened tokens
OC = 512                  # g-matmul output chunk
NOC = (D * D) // OC       # 8 chunks per head
DPC = OC // D             # 8 d-values per chunk
NTT = T // 128            # 32 token tiles
NKT = E // 128            # 4 contraction tiles
QVKW = 256                # fused q|v_aug|k projection width (zero padded)
M = 8                     # cores

_cache = {}


def _build():
    if "nc" in _cache:
        return _cache["nc"]
    from contextlib import ExitStack

    import concourse.bass as bass
    import concourse.bacc as bacc
    import concourse.mybir as mybir
    import concourse.tile as tile
    from concourse.masks import make_identity

    F32 = mybir.dt.float32
    F32R = mybir.dt.float32r
    BF16 = mybir.dt.bfloat16
    AF = mybir.ActivationFunctionType
    ALU = mybir.AluOpType
    AX = mybir.AxisListType

    nc = bacc.Bacc(trn_type="TRN2")
    xT_d = nc.dram_tensor("xT", [E, T], F32R, kind="ExternalInput")
    Wg_d = nc.dram_tensor("Wg", [E, D * D], F32R, kind="ExternalInput")
    bg_d = nc.dram_tensor("bg", [1, D * D], BF16, kind="ExternalInput")
    Wqvk_d = nc.dram_tensor("Wqvk", [E, QVKW], F32R, kind="ExternalInput")
    bqvk_d = nc.dram_tensor("bqvk", [1, QVKW], BF16, kind="ExternalInput")
    outT_d = nc.dram_tensor("outT", [D + 1, T], F32, kind="ExternalOutput")

    with tile.TileContext(nc) as tc, ExitStack() as ctx:
        const = ctx.enter_context(tc.tile_pool(name="const", bufs=1))
        acts = ctx.enter_context(tc.tile_pool(name="acts", bufs=1))

        wqvk_sb = []
        for kt in range(NKT):
            wqt = const.tile([128, QVKW], F32R, tag=f"wqvk{kt}")
            nc.sync.dma_start(wqt[:], Wqvk_d[kt * 128:(kt + 1) * 128, :])
            wqvk_sb.append(wqt)
        bg_sb = const.tile([1, D * D], BF16)
        nc.sync.dma_start(bg_sb[:], bg_d[:, :])
        bqvk_sb = const.tile([1, QVKW], BF16)
        nc.sync.dma_start(bqvk_sb[:], bqvk_d[:, :])
        ones32 = const.tile([1, 128], F32)
        nc.gpsimd.memset(ones32[:], 1.0)
        ones_b = const.tile([1, 128], BF16)
        nc.gpsimd.tensor_copy(ones_b[:], ones32[:])
        ident = const.tile([128, 128], F32)
        make_identity(nc, ident[:])

        # persistent per-head activations
        q_sb = acts.tile([128, NTT, D], F32)       # q, natural layout
        v_sb = acts.tile([128, NTT, D + 1], F32R)  # v | ones column
        kT_sb = acts.tile([D, T], F32R)
        qgT_sb = acts.tile([D, T], F32R)

        # ---------------- phase 1: projections, g, qg ----------------
        with ExitStack() as p1:
            xpool = p1.enter_context(tc.tile_pool(name="xp", bufs=2))
            wgpool = p1.enter_context(tc.tile_pool(name="wgp", bufs=1))
            wg_sb = []
            for kt in range(NKT):
                wgt = wgpool.tile([128, D * D], F32R, tag=f"wg{kt}",
                                  name=f"wg{kt}")
                wg_sb.append(wgt)
            QL = (D * D) // 4
            for quar in range(4):
                for kt in range(NKT):
                    nc.scalar.dma_start(
                        wg_sb[kt][:, quar * QL:(quar + 1) * QL],
                        Wg_d[kt * 128:(kt + 1) * 128,
                             quar * QL:(quar + 1) * QL])
            gpool = p1.enter_context(tc.tile_pool(name="gp", bufs=5))
            dpool = p1.enter_context(tc.tile_pool(name="dp", bufs=4))
            pp_g = p1.enter_context(
                tc.tile_pool(name="pg", bufs=7, space="PSUM"))
            pp_qvk = pp_g
            pp_tr = p1.enter_context(
                tc.tile_pool(name="ptr", bufs=1, space="PSUM"))

            pending = []  # (tc0, k_nat, qg_t) awaiting PE transpose

            def flush_pending():
                for ptc0, pk, pqg in pending:
                    ptr = pp_tr.tile([D, 128], F32, tag="tr", name="ktr")
                    nc.tensor.transpose(ptr[:], pk[:], ident[:])
                    nc.vector.tensor_copy(kT_sb[:, ptc0:ptc0 + 128], ptr[:])
                    ptr2 = pp_tr.tile([D, 128], F32, tag="tr", name="qgtr")
                    nc.tensor.transpose(ptr2[:], pqg[:], ident[:])
                    nc.vector.tensor_copy(qgT_sb[:, ptc0:ptc0 + 128], ptr2[:])
                pending.clear()

            for tt in range(NTT):
                tc0 = tt * 128
                xs = []
                for kt in range(NKT):
                    xt = xpool.tile([128, 128], F32R, tag=f"x{kt}")
                    nc.sync.dma_start(
                        xt[:], xT_d[kt * 128:(kt + 1) * 128, tc0:tc0 + 128])
                    xs.append(xt)

                # two rounds of 4 g-chunks; round 0 also carries the qvk
                # projection so each (round, kt) is a same-lhsT matmul run
                pq = pp_qvk.tile([128, OC], F32, tag="pg", name="pq")
                pgs = {}
                for rnd in range(2):
                    for kt in range(NKT):
                        if rnd == 0:
                            nc.tensor.matmul(pq[:, 0:QVKW], xs[kt][:],
                                             wqvk_sb[kt][:],
                                             start=(kt == 0), stop=False)
                        for oc in range(rnd * 4, rnd * 4 + 4):
                            oc0 = oc * OC
                            if kt == 0:
                                pgs[oc] = pp_g.tile([128, OC], F32, tag="pg", name=f"pg{oc}")
                            nc.tensor.matmul(
                                pgs[oc][:], xs[kt][:],
                                wg_sb[kt][:, oc0:oc0 + OC],
                                start=(kt == 0), stop=False)
                    if rnd == 0:
                        nc.tensor.matmul(pq[:, 0:QVKW], ones_b[:],
                                         bqvk_sb[:], start=False, stop=True)
                    for oc in range(rnd * 4, rnd * 4 + 4):
                        oc0 = oc * OC
                        nc.tensor.matmul(pgs[oc][:], ones_b[:],
                                         bg_sb[:, oc0:oc0 + OC],
                                         start=False, stop=True)
                    if rnd == 0:
                        flush_pending()  # prior tile's transposes mid-stream

                nc.vector.tensor_copy(q_sb[:, tt, :], pq[:, 0:D])
                nc.vector.tensor_copy(v_sb[:, tt, :], pq[:, D:2 * D + 1])
                k_nat = dpool.tile([128, D], F32, tag="knat")
                nc.vector.tensor_copy(k_nat[:], pq[:, 2 * D + 1:3 * D + 1])

                # gelu + qg contraction per chunk
                qg_part = dpool.tile([128, NOC, D], F32, tag="qgp")
                for oc in range(NOC):
                    # gelu, written (e-major, d-minor) so the d-reduce is
                    # contiguous
                    g_t = gpool.tile([128, OC], F32, tag="g")
                    gw = g_t[:]
                    g_ed = bass.AP(tensor=gw.tensor, offset=gw.offset,
                                   ap=[gw.ap[0], [1, DPC], [DPC, D]])
                    nc.scalar.activation(g_ed, pgs[oc][:], AF.Gelu)
                    prod = gpool.tile([128, OC], F32, tag="prod")
                    qs = q_sb[:, tt, :]
                    q3 = bass.AP(
                        tensor=qs.tensor,
                        offset=qs.offset + oc * DPC,
                        ap=[qs.ap[0], [0, D], [1, DPC]])
                    nc.vector.tensor_tensor(
                        prod[:].rearrange("p (e d) -> p e d", d=DPC),
                        g_t[:].rearrange("p (e d) -> p e d", d=DPC),
                        q3, op=ALU.mult)
                    nc.vector.tensor_reduce(
                        qg_part[:, oc, :],
                        prod[:].rearrange("p (e d) -> p e d", d=DPC),
                        axis=AX.X, op=ALU.add)
                qg_t = dpool.tile([128, D], F32, tag="qg")
                qp = qg_part[:]
                qpv = bass.AP(tensor=qp.tensor, offset=qp.offset,
                              ap=[qp.ap[0], [1, D], [D, NOC]])
                nc.vector.tensor_reduce(qg_t[:], qpv, axis=AX.X, op=ALU.add)
                pending.append((tc0, k_nat, qg_t))
            flush_pending()

        # ---------------- phase 2: attention ----------------
        with ExitStack() as p2:
            espool = p2.enter_context(tc.tile_pool(name="es", bufs=34))
            outp = p2.enter_context(tc.tile_pool(name="outp", bufs=4))
            pp_s = p2.enter_context(
                tc.tile_pool(name="psc", bufs=6, space="PSUM"))
            pp_av = p2.enter_context(
                tc.tile_pool(name="pav", bufs=2, space="PSUM"))

            NMT = N // 128  # m tiles per batch
            NNC = N // OC   # n chunks per batch
            pending_av = []  # (b, nch, es-dict) awaiting av emission

            def emit_av():
                if not pending_av:
                    return
                by_b = {}
                for bb, nch, esd in pending_av:
                    by_b.setdefault(bb, {})[nch] = esd
                for bb, chunks in by_b.items():
                    pavs = {nch: pp_av.tile([D + 1, OC], F32, tag="av",
                                            name=f"pav{nch}")
                            for nch in chunks}
                    for mt in range(NMT):
                        for nch, esd in chunks.items():
                            nc.tensor.matmul(pavs[nch][:],
                                             v_sb[:, bb * NMT + mt, :],
                                             esd[mt][:],
                                             start=(mt == 0),
                                             stop=(mt == NMT - 1))
                    for nch in chunks:
                        nc0 = bb * N + nch * OC
                        o_t = outp.tile([D + 1, OC], F32, tag="o", name="o_t")
                        nc.vector.tensor_copy(o_t[:], pavs[nch][:])
                        nc.sync.dma_start(outT_d[:, nc0:nc0 + OC], o_t[:])
                pending_av.clear()

            for b in range(B):
                es = {}
                # S^T and exp for the whole batch; kT slice (lhsT) is
                # reused across both n-chunks
                for mt in range(NMT):
                    if mt == 2:
                        emit_av()  # prior batch's av, mid-stream
                    mc0 = b * N + mt * 128
                    for nch in range(NNC):
                        nc0 = b * N + nch * OC
                        ps_ = pp_s.tile([128, OC], F32, tag="s")
                        nc.tensor.matmul(ps_[:], kT_sb[:, mc0:mc0 + 128],
                                         qgT_sb[:, nc0:nc0 + OC],
                                         start=True, stop=True)
                        e_t = espool.tile([128, OC], F32R, tag="es")
                        nc.scalar.activation(e_t[:], ps_[:], AF.Exp)
                        es[(mt, nch)] = e_t
                for nch in range(NNC):
                    pending_av.append((b, nch, {mt: es[(mt, nch)]
                                                for mt in range(NMT)}))
            emit_av()

    nc.compile()
    _cache["nc"] = nc
    return nc


def _make_in_maps(x, Wq, bq, Wk, bk, Wv, bv, Wg, bg):
    import ml_dtypes
    x = np.asarray(x, dtype=np.float32)
    xT = np.ascontiguousarray(x.reshape(T, E).T)
    in_maps = []
    for h in range(M):
        c0 = h * D
        Wqvk = np.zeros((E, QVKW), dtype=np.float32)
        Wqvk[:, 0:D] = Wq[:, c0:c0 + D]
        Wqvk[:, D:2 * D] = Wv[:, c0:c0 + D]
        # column 2*D is the ones column of v_aug: weight 0, bias 1
        Wqvk[:, 2 * D + 1:3 * D + 1] = Wk[:, c0:c0 + D]
        bqvk = np.zeros((1, QVKW), dtype=np.float32)
        bqvk[0, 0:D] = bq[c0:c0 + D]
        bqvk[0, D:2 * D] = bv[c0:c0 + D]
        bqvk[0, 2 * D] = 1.0
        bqvk[0, 2 * D + 1:3 * D + 1] = bk[c0:c0 + D]
        g0 = h * D * D
        in_maps.append(dict(
            xT=xT,
            Wg=np.ascontiguousarray(Wg[:, g0:g0 + D * D], dtype=np.float32),
            bg=np.ascontiguousarray(bg[g0:g0 + D * D], dtype=np.float32)
            .reshape(1, D * D).astype(ml_dtypes.bfloat16),
            Wqvk=Wqvk,
            bqvk=bqvk.astype(ml_dtypes.bfloat16),
        ))
    return in_maps


def kernel(x, Wq, bq, Wk, bk, Wv, bv, Wg, bg):
    from concourse import bass_utils

    nc = _build()
    in_maps = _make_in_maps(x, Wq, bq, Wk, bk, Wv, bv, Wg, bg)
    res = bass_utils.run_bass_kernel_spmd(nc, in_maps, core_ids=list(range(M)))
    out = np.empty((B, N, H, D), dtype=np.float32)
    for h in range(M):
        oT = res.results[h]["outT"]           # [65, T]
        o = (oT[:D] / oT[D:D + 1]).T          # [T, 64]
        out[:, :, h, :] = o.reshape(B, N, D)
    return out.reshape(B, N, E)



# revision 2
# speedup vs baseline: 1.2057x; 1.2057x over previous
"""Fused OOQKV attention-with-generated-transform kernel for Trainium2 (v5).

Math (per head h, one head per core):
  g = gelu(x @ Wg_h + bg_h)            # [T, 64, 64] per-token transform
  q,k,v = x @ W{q,k,v}_h + b           # [T, 64]
  qg[t] = q[t] @ g[t]
  att = softmax(qg @ k^T)              # per batch, no scaling
  out_h = att @ v

Key layout/engine choices (all measured on HW):
  - All matmul inputs fp16 (1 cycle/row streaming like f32r, half DMA,
    cheap LDWEIGHTS).  PSUM accumulation stays f32.
  - Wg columns host-permuted so each 512-chunk is (e-major, dsub-minor):
    gelu writes contiguously; chunk pairs share a [128,1024] 2-bank PSUM
    tile so one ACT instruction covers 1024 elems.
  - g-bias: 3 pairs via K=1 ones-matmuls on PE, 1 pair via a DVE
    tensor-add against a broadcast bias tile (balances PE vs DVE load).
  - q/v biases ride the PSUM->SBUF extraction adds on DVE (no bias
    matmul, no ones column in the projection; v's softmax-denominator
    ones column is memset once).
  - qg contraction on DVE in packed fp16 (2x mode): per pair one mult
    against a strided q broadcast view, add-tree, halves-add, one reduce.
  - k is produced TRANSPOSED by stationary-Wk matmuls (bias via ACT
    per-partition bias) grouped late in phase 1; batch 0/1 score bursts
    also interleave there so the attention tail is short.
  - Phase 2 pipelines S^T/exp/AV across batches; es and v are bf16 so
    three batches of staged exp() fit in SBUF.
  - Few, large DMAs: the DMA-trigger rate (~1us/trigger on a sequencer)
    dominated startup with many small transfers.
"""

import sys

sys.path.insert(0, "/opt/trn_rl_repo")

import numpy as np

B, N, E, H, D = 4, 1024, 512, 8, 64
T = B * N                 # 4096 flattened tokens
NTT = T // 128            # 32 token tiles
NKT = E // 128            # 4 contraction tiles
PKW = 3 * D               # packed q|v|k weight width per kt
M = 8                     # cores

_cache = {}


def _build():
    if "nc" in _cache:
        return _cache["nc"]
    from contextlib import ExitStack

    import concourse.bass as bass
    import concourse.bacc as bacc
    import concourse.mybir as mybir
    import concourse.tile as tile
    from concourse.masks import make_identity

    F32 = mybir.dt.float32
    F32R = mybir.dt.float32r
    F16 = mybir.dt.float16
    BF16 = mybir.dt.bfloat16
    AF = mybir.ActivationFunctionType
    ALU = mybir.AluOpType
    AX = mybir.AxisListType

    nc = bacc.Bacc(trn_type="TRN2")
    xT_d = nc.dram_tensor("xT", [E, T], F16, kind="ExternalInput")
    Wg_d = nc.dram_tensor("Wg", [E, D * D], F16, kind="ExternalInput")
    # [128, 4*192]: per-partition packed (Wq | Wv | Wk) for each kt
    Wp_d = nc.dram_tensor("Wp", [128, NKT * PKW], F16, kind="ExternalInput")
    # [1, 128 + 4096]: (bq | bv) | bg(permuted)
    bp_d = nc.dram_tensor("bp", [1, 2 * D + D * D], F16, kind="ExternalInput")
    bk_d = nc.dram_tensor("bk", [D, 1], F32, kind="ExternalInput")
    outT_d = nc.dram_tensor("outT", [D + 1, T], F32, kind="ExternalOutput")

    with tile.TileContext(nc) as tc, ExitStack() as ctx:
        const = ctx.enter_context(tc.tile_pool(name="const", bufs=1))
        acts = ctx.enter_context(tc.tile_pool(name="acts", bufs=1))
        q_pool = ctx.enter_context(tc.tile_pool(name="qp", bufs=10))
        gpool = ctx.enter_context(tc.tile_pool(name="gp", bufs=4))
        zpool = ctx.enter_context(tc.tile_pool(name="zp", bufs=2))
        apool = ctx.enter_context(tc.tile_pool(name="ap", bufs=9))
        ppool = ctx.enter_context(tc.tile_pool(name="pp", bufs=3))
        hpool = ctx.enter_context(tc.tile_pool(name="hp", bufs=10))
        espool = ctx.enter_context(tc.tile_pool(name="es", bufs=16))
        outp = ctx.enter_context(tc.tile_pool(name="outp", bufs=4))
        pairs = ctx.enter_context(
            tc.tile_pool(name="pgp", bufs=3, space="PSUM"))
        pqtr = ctx.enter_context(
            tc.tile_pool(name="pqtr", bufs=1, space="PSUM"))

        # ---- constants / weights (3 packed DMAs on the sync queue) ----
        wp_sb = const.tile([128, NKT * PKW], F16)
        nc.sync.dma_start(wp_sb[:], Wp_d[:, :])
        bp_sb = const.tile([1, 2 * D + D * D], F16)
        nc.sync.dma_start(bp_sb[:], bp_d[:, :])
        bk_sb = const.tile([D, 1], F32)
        nc.sync.dma_start(bk_sb[:], bk_d[:, :])
        wqv_sb = [wp_sb[:, kt * PKW:kt * PKW + 2 * D] for kt in range(NKT)]
        wk_sb = [wp_sb[:, kt * PKW + 2 * D:(kt + 1) * PKW]
                 for kt in range(NKT)]
        bg_sb = bp_sb[:, 2 * D:]
        ones32 = const.tile([1, 128], F32)
        nc.vector.memset(ones32[:], 1.0)
        ones16 = const.tile([1, 128], F16)
        nc.vector.tensor_copy(ones16[:], ones32[:])
        ident = const.tile([128, 128], F16)
        make_identity(nc, ident[:])

        # broadcast bias tiles (one-time, via ones-matmuls through PSUM)
        qvb_bc = const.tile([128, 2 * D], F32)
        gb3_bc = const.tile([128, 1024], F32)
        binit = pairs.tile([128, 1024], F32, tag="pgpair", name="binit")
        nc.tensor.matmul(binit[:, 0:2 * D], ones16[:], bp_sb[:, 0:2 * D],
                         start=True, stop=True)
        nc.vector.tensor_copy(qvb_bc[:], binit[:, 0:2 * D])
        binit2 = pairs.tile([128, 1024], F32, tag="pgpair", name="binit2")
        for half in range(2):
            nc.tensor.matmul(binit2[:, half * 512:half * 512 + 512],
                             ones16[:], bg_sb[:, 3072 + half * 512:
                                              3072 + half * 512 + 512],
                             start=True, stop=True)
        nc.vector.tensor_copy(gb3_bc[:], binit2[:])

        # ---- resident xT: chunk 0 (tiles 0-3), then the rest ----
        xt_sb = []
        for kt in range(NKT):
            xt = acts.tile([128, T], F16, tag=f"xt{kt}", name=f"xt{kt}")
            xt_sb.append(xt)
        for kt in range(NKT):
            nc.gpsimd.dma_start(xt_sb[kt][:, 0:512],
                                xT_d[kt * 128:(kt + 1) * 128, 0:512])
        for kt in range(NKT):
            nc.sync.dma_start(xt_sb[kt][:, 512:1024],
                              xT_d[kt * 128:(kt + 1) * 128, 512:1024])

        # ---- resident Wg: 8 half-transfers on the scalar queue ----
        wg_sb = []
        for kt in range(NKT):
            wgt = acts.tile([128, D * D], F16, tag=f"wg{kt}", name=f"wg{kt}")
            wg_sb.append(wgt)
        def wg_eng(q, kt):
            if kt in (0, 1):
                return nc.scalar
            if kt == 2:
                return nc.gpsimd if q < 2 else nc.sync
            return nc.gpsimd
        for q in range(4):
            for kt in range(NKT):
                wg_eng(q, kt).dma_start(
                    wg_sb[kt][:, q * 1024:(q + 1) * 1024],
                    Wg_d[kt * 128:(kt + 1) * 128, q * 1024:(q + 1) * 1024])

        # remaining xT on the gpsimd trigger queue
        for kt in range(NKT):
            nc.gpsimd.dma_start(xt_sb[kt][:, 1024:T],
                                xT_d[kt * 128:(kt + 1) * 128, 1024:T])

        # ---- persistent per-head activations ----
        v_sb = acts.tile([128, NTT, D + 1], BF16)
        vw = v_sb[:]
        vones = bass.AP(tensor=vw.tensor, offset=vw.offset + D,
                        ap=[vw.ap[0], [D + 1, NTT]])
        nc.gpsimd.memset(vones, 1.0)  # softmax-denominator ones column
        kT_sb = acts.tile([D, T], F16)
        qgT_sb = acts.tile([D, T], F16)

        NMT = N // 128  # m tiles per batch
        pending = []    # (tc0, qg_t) awaiting PE transpose + copy
        state = {}      # tt -> dict(q_t, gts, acc)
        es_all = {}     # b -> list of es pair tiles

        def flush_pending(upto):
            while len(pending) > upto:
                ptc0, pqg = pending.pop(0)
                ptr = pqtr.tile([D, 128], F16, tag="tr", name="ptr")
                nc.tensor.transpose(ptr[:], pqg[:], ident[:])
                nc.vector.tensor_copy(qgT_sb[:, ptc0:ptc0 + 128], ptr[:])

        def mult_add(st, j):
            # DVE: P_j = g~_j * q[:, 16j:16j+16] (bcast view); acc += P_j
            qw = st["q_t"][:]
            q3 = bass.AP(tensor=qw.tensor, offset=qw.offset + j * 16,
                         ap=[qw.ap[0], [8, 2], [0, D], [1, 8]])
            gv = st["gts"][j][:].rearrange("p (h e d) -> p h e d", h=2, d=8)
            with nc.allow_low_precision(reason="fp16 qg accumulation"):
                if j == 0:
                    nc.vector.tensor_tensor(
                        st["acc"][:].rearrange("p (h e d) -> p h e d",
                                               h=2, d=8),
                        gv, q3, op=ALU.mult)
                else:
                    prod = ppool.tile([128, 1024], F16, tag="prod",
                                      name="prod")
                    nc.vector.tensor_tensor(
                        prod[:].rearrange("p (h e d) -> p h e d", h=2, d=8),
                        gv, q3, op=ALU.mult)
                    nc.vector.tensor_tensor(st["acc"][:], st["acc"][:],
                                            prod[:], op=ALU.add)

        def stage_a(tt):
            tc0 = tt * 128
            xs = [xt_sb[kt][:, tc0:tc0 + 128] for kt in range(NKT)]
            st = state[tt] = {}
            pq = pqtr.tile([128, 2 * D], F32, tag="pq", name="pq")
            prs = []
            for j in range(2):
                pr = pairs.tile([128, 1024], F32, tag="pgpair",
                                name=f"prA{j}")
                prs.append(pr)
            for kt in range(NKT):
                nc.tensor.matmul(pq[:], xs[kt], wqv_sb[kt],
                                 start=(kt == 0), stop=(kt == NKT - 1))
                for j in range(2):
                    for half in range(2):
                        c0 = j * 1024 + half * 512
                        nc.tensor.matmul(
                            prs[j][:, half * 512:half * 512 + 512],
                            xs[kt], wg_sb[kt][:, c0:c0 + 512],
                            start=(kt == 0), stop=False)
            for j in range(2):
                for half in range(2):
                    c0 = j * 1024 + half * 512
                    nc.tensor.matmul(
                        prs[j][:, half * 512:half * 512 + 512],
                        ones16[:], bg_sb[:, c0:c0 + 512],
                        start=False, stop=True)
            flush_pending(1)
            gts = []
            for j in range(2):
                g_t = gpool.tile([128, 1024], F16, tag="g", name=f"gA{j}")
                nc.scalar.activation(g_t[:], prs[j][:], AF.Gelu)
                gts.append(g_t)
            st["gts"] = gts
            # q/v extraction with fused bias adds (DVE)
            q_t = q_pool.tile([128, D], F16, tag="q", name="q_t")
            with nc.allow_low_precision(reason="fp16 q"):
                nc.vector.tensor_tensor(q_t[:], pq[:, 0:D], qvb_bc[:, 0:D],
                                        op=ALU.add)
                nc.vector.tensor_tensor(v_sb[:, tt, 0:D], pq[:, D:2 * D],
                                        qvb_bc[:, D:2 * D], op=ALU.add)
            st["q_t"] = q_t
            st["acc"] = apool.tile([128, 1024], F16, tag="acc", name="acc")
            mult_add(st, 0)
            mult_add(st, 1)

        def stage_b(tt):
            tc0 = tt * 128
            xs = [xt_sb[kt][:, tc0:tc0 + 128] for kt in range(NKT)]
            st = state[tt]
            prs = []
            for j in range(2):
                pr = pairs.tile([128, 1024], F32, tag="pgpair",
                                name=f"prB{j}")
                prs.append(pr)
            for kt in range(NKT):
                for j in range(2):
                    for half in range(2):
                        c0 = 2048 + j * 1024 + half * 512
                        nc.tensor.matmul(
                            prs[j][:, half * 512:half * 512 + 512],
                            xs[kt], wg_sb[kt][:, c0:c0 + 512],
                            start=(kt == 0),
                            stop=(j == 1 and kt == NKT - 1))
            # pair B0 bias on PE; pair B1 bias on DVE via broadcast add
            for half in range(2):
                c0 = 2048 + half * 512
                nc.tensor.matmul(prs[0][:, half * 512:half * 512 + 512],
                                 ones16[:], bg_sb[:, c0:c0 + 512],
                                 start=False, stop=True)
            g_t0 = gpool.tile([128, 1024], F16, tag="g", name="gB0")
            nc.scalar.activation(g_t0[:], prs[0][:], AF.Gelu)
            st["gts"].append(g_t0)
            z_t = zpool.tile([128, 1024], F16, tag="z", name="z_t")
            with nc.allow_low_precision(reason="fp16 z"):
                nc.vector.tensor_tensor(z_t[:], prs[1][:], gb3_bc[:],
                                        op=ALU.add)
            g_t1 = gpool.tile([128, 1024], F16, tag="g", name="gB1")
            nc.scalar.activation(g_t1[:], z_t[:], AF.Gelu)
            st["gts"].append(g_t1)
            mult_add(st, 2)
            mult_add(st, 3)
            with nc.allow_low_precision(reason="fp16 qg accumulation"):
                accH = hpool.tile([128, 512], F16, tag="accH", name="accH")
                nc.vector.tensor_tensor(accH[:], st["acc"][:, 0:512],
                                        st["acc"][:, 512:1024], op=ALU.add)
                qg_t = hpool.tile([128, D], F16, tag="qg", name="qg_t")
                nc.vector.tensor_reduce(
                    qg_t[:], accH[:].rearrange("p (e d) -> p e d", d=8),
                    axis=AX.X, op=ALU.add)
            pending.append((tc0, qg_t))
            del state[tt]["gts"]

        def k_pass():
            # kT via stationary Wk; identities grouped (one table switch)
            kp_tiles = []
            for c in range(8):
                kp = pairs.tile([128, 1024], F32, tag="pgpair",
                                name=f"kp{c % 2}")
                for kt in range(NKT):
                    nc.tensor.matmul(kp[0:D, 0:512], wk_sb[kt],
                                     xt_sb[kt][:, c * 512:(c + 1) * 512],
                                     start=(kt == 0), stop=(kt == NKT - 1))
                nc.scalar.activation(kT_sb[:, c * 512:(c + 1) * 512],
                                     kp[0:D, 0:512], AF.Identity,
                                     bias=bk_sb[:])
                kp_tiles.append(kp)

        def s_burst(b):
            esl = es_all[b] = []
            for mt in range(NMT):
                mc0 = b * N + mt * 128
                ps_ = pairs.tile([128, 1024], F32, tag="pgpair", name="ps_")
                for nch in range(2):
                    nc.tensor.matmul(
                        ps_[:, nch * 512:nch * 512 + 512],
                        kT_sb[:, mc0:mc0 + 128],
                        qgT_sb[:, b * N + nch * 512:b * N + nch * 512 + 512],
                        start=True, stop=True)
                e_t = espool.tile([128, 1024], BF16, tag="es", name="e_t")
                nc.scalar.activation(e_t[:], ps_[:], AF.Exp)
                esl.append(e_t)

        def av_burst(b):
            esl = es_all.pop(b)
            for nch in range(2):
                pav = pairs.tile([128, 1024], F32, tag="pgpair", name="pav")
                for mt in range(NMT):
                    nc.tensor.matmul(
                        pav[0:D + 1, 0:512], v_sb[:, b * NMT + mt, :],
                        esl[mt][:, nch * 512:nch * 512 + 512],
                        start=(mt == 0), stop=(mt == NMT - 1))
                nc0 = b * N + nch * 512
                o_t = outp.tile([D + 1, 512], F32, tag="o", name="o_t")
                nc.vector.tensor_copy(o_t[:], pav[0:D + 1, 0:512])
                nc.sync.dma_start(outT_d[:, nc0:nc0 + 512], o_t[:])

        for tt in range(8):
            stage_a(tt)
        for tt in range(8):
            stage_b(tt)
        for tt in range(8, NTT):
            stage_a(tt)
            stage_b(tt)
            if tt == 25:
                k_pass()
            elif tt == 27:
                s_burst(0)
            elif tt == 29:
                s_burst(1)
            elif tt == 30:
                av_burst(0)
        flush_pending(0)
        s_burst(2)
        av_burst(1)
        s_burst(3)
        av_burst(2)
        av_burst(3)

    nc.compile()
    _cache["nc"] = nc
    return nc


def _make_in_maps(x, Wq, bq, Wk, bk, Wv, bv, Wg, bg):
    x = np.asarray(x, dtype=np.float32)
    xT = np.ascontiguousarray(x.reshape(T, E).T.astype(np.float16))
    in_maps = []
    for h in range(M):
        c0 = h * D
        # Wg columns permuted: chunk oc of 512 cols reordered (e, dsub)
        Wgh = np.asarray(Wg[:, h * D * D:(h + 1) * D * D], dtype=np.float32)
        Wgp = np.ascontiguousarray(
            Wgh.reshape(E, 8, 8, D).transpose(0, 1, 3, 2).reshape(E, D * D)
            .astype(np.float16))
        bgh = np.asarray(bg[h * D * D:(h + 1) * D * D], dtype=np.float32)
        bgp = (bgh.reshape(8, 8, D).transpose(0, 2, 1).reshape(D * D)
               .astype(np.float16))
        # packed per-kt (Wq | Wv | Wk): [128, 4*192]
        Wp = np.zeros((128, NKT * PKW), dtype=np.float16)
        for kt in range(NKT):
            r0 = kt * 128
            Wp[:, kt * PKW:kt * PKW + D] = Wq[r0:r0 + 128, c0:c0 + D]
            Wp[:, kt * PKW + D:kt * PKW + 2 * D] = Wv[r0:r0 + 128,
                                                      c0:c0 + D]
            Wp[:, kt * PKW + 2 * D:(kt + 1) * PKW] = Wk[r0:r0 + 128,
                                                        c0:c0 + D]
        bp = np.zeros((1, 2 * D + D * D), dtype=np.float16)
        bp[0, 0:D] = bq[c0:c0 + D]
        bp[0, D:2 * D] = bv[c0:c0 + D]
        bp[0, 2 * D:] = bgp
        in_maps.append(dict(
            xT=xT,
            Wg=Wgp,
            Wp=Wp,
            bp=bp,
            bk=np.ascontiguousarray(
                np.asarray(bk[c0:c0 + D], np.float32).reshape(D, 1)),
        ))
    return in_maps


def kernel(x, Wq, bq, Wk, bk, Wv, bv, Wg, bg):
    from concourse import bass_utils

    nc = _build()
    in_maps = _make_in_maps(x, Wq, bq, Wk, bk, Wv, bv, Wg, bg)
    _cache["in_maps"] = in_maps
    res = bass_utils.run_bass_kernel_spmd(nc, in_maps, core_ids=list(range(M)))
    out = np.empty((B, N, H, D), dtype=np.float32)
    for h in range(M):
        oT = res.results[h]["outT"]           # [65, T]
        o = (oT[:D] / oT[D:D + 1]).T          # [T, 64]
        out[:, :, h, :] = o.reshape(B, N, D)
    return out.reshape(B, N, E)
